# revision 29
# baseline (speedup 1.0000x reference)
"""Trainium2 Bass kernel for multi-head attention (B=4, N=2048, DIM=1024, H=16, DH=64).

Sharding (head-parallel + row-parallel to_out): 8 cores = 4 batches x 2 head-halves.
Each core computes q/k/v for its 8 heads over the FULL 2048-token sequence, runs
attention for those heads, and its row-parallel half of the output projection. The
to_out all-reduce happens on the host at gather time:
out[b] = partial[core 2b] + partial[core 2b+1] + bias.

The attention phase is ACT(exp)-bound: softmax exp runs only on the scalar engine at
1 elem/cycle/lane, so the per-core floor is 256 x [128,1024]-exp = ~294 us. This
kernel keeps the ACT stream contiguous from ~18us on by:
  * blocks of (head-pair s, 512-query block qb): both heads' scores live side by
    side in ONE [128,1024] fp32 PSUM tile (2 banks) -> one exp instruction per
    key-tile step covers both heads at full FD=1024 efficiency.
  * score matmuls for the two heads are issued back-to-back into disjoint PE row
    groups (tile_position (0,0)/(64,0)) so they stream concurrently (~2x).
  * av accumulators are [65,512] (1 PSUM bank each, ones-column denominator), so
    PSUM = 4(sc double-buffer) + 2(av) + 2(util) banks, leaving a util pool for
    projection matmuls to interleave with attention.
  * input DMAs are chunked and priority-ordered (first key/query weight slices,
    x column blocks in consumption order), so the first exp fires at ~18us; the
    qkv projections, late q chunks, and the output projection stream into the
    per-step PE slack via a budget-tracked filler queue.
  * a ~44-matmul garbage warm-up burst keeps the PE HAM clock-gate busy across
    the initial DMA wait so projection matmuls run at 2.4 GHz from the start.
Softmax denominators are folded into the AV matmul via a ones-column in V; the
reciprocal is spread over 64 partitions via an SBUF DMA restructure, then
broadcast back through a DRAM round-trip.
"""

import numpy as np
import ml_dtypes

import concourse.bass as bass
import concourse.tile as tile
from concourse import bacc, mybir
from concourse import bass_utils

B, N, DIM = 4, 2048, 1024
HEADS_TOT, DH = 16, 64
SCALE = DH ** -0.5
NCORES = 8

HPC = 8              # heads per core
NPAIR = HPC // 2     # head-pairs per core = 4
KT = DIM // 128      # 8 contraction tiles
NT = N // 128        # 16 key tiles
QB = 512             # queries per block
NQB = N // QB        # 4 query blocks
BF16 = mybir.dt.bfloat16
F32 = mybir.dt.float32

_CACHE = {}


def _build_program():
    nc = bacc.Bacc("TRN2", target_bir_lowering=False, debug=False)

    xT_d = nc.dram_tensor("xT", [128, NQB, KT, 512], BF16, kind="ExternalInput")
    w_d = nc.dram_tensor("w_qkv", [128, 3, KT, 512], BF16, kind="ExternalInput")
    wo_d = nc.dram_tensor("w_out", [128, NPAIR, DIM], BF16, kind="ExternalInput")
    out_d = nc.dram_tensor("out", [N, DIM], BF16, kind="ExternalOutput")

    with tile.TileContext(nc) as tc:
        _emit(tc, nc, xT_d, w_d, wo_d, out_d)
    nc.compile()
    return nc


def _emit(tc, nc, xT_d, w_d, wo_d, out_d):
    from contextlib import ExitStack

    with ExitStack() as ctx:
        consts = ctx.enter_context(tc.tile_pool(name="consts", bufs=1))
        stage = ctx.enter_context(tc.tile_pool(name="stage", bufs=1))
        qkv = ctx.enter_context(tc.tile_pool(name="qkv", bufs=1))
        ao = ctx.enter_context(tc.tile_pool(name="ao", bufs=1))
        atp = ctx.enter_context(tc.tile_pool(name="atp", bufs=6))
        avup = ctx.enter_context(tc.tile_pool(name="avu", bufs=2))
        rcp = ctx.enter_context(tc.tile_pool(name="rcp", bufs=2))
        bcsp = ctx.enter_context(tc.tile_pool(name="bcs", bufs=2))
        oddp = ctx.enter_context(tc.tile_pool(name="odd", bufs=2))
        drbp = ctx.enter_context(tc.tile_pool(name="drb", bufs=2, space="DRAM"))
        stp = ctx.enter_context(tc.tile_pool(name="stp", bufs=2))
        stpt = ctx.enter_context(tc.tile_pool(name="stt", bufs=8))
        scp = ctx.enter_context(tc.tile_pool(name="scp", bufs=2, space="PSUM"))
        avp = ctx.enter_context(tc.tile_pool(name="avp", bufs=2, space="PSUM"))
        utilp = ctx.enter_context(tc.tile_pool(name="utl", bufs=2, space="PSUM"))

        # ---- persistent SBUF tiles ----
        wo_sb = consts.tile([128, NPAIR, DIM], BF16)
        at_warm = consts.tile([128, 32], BF16)
        wv = stage.tile([128, KT, 512], BF16)
        wk = stage.tile([128, KT, 512], BF16)
        wq = stage.tile([128, KT, 512], BF16)
        xc = [stage.tile([128, KT, 512], BF16, name=f"xc{c}") for c in range(NQB)]
        kTs = [qkv.tile([128, N], BF16, name=f"kT{s}") for s in range(NPAIR)]
        qTs = [qkv.tile([128, N], BF16, name=f"qT{s}") for s in range(NPAIR)]
        vt = qkv.tile([128, NT, HPC, DH + 1], BF16)
        aoTs = [ao.tile([128, N], BF16, name=f"aoT{s}") for s in range(NPAIR)]

        # ---- exp table preload (reads garbage SBUF; off the data critical path)
        nc.scalar.activation(out=at_warm, in_=kTs[0][:, 0:32],
                             func=mybir.ActivationFunctionType.Exp)

        # ---- input DMAs: chunked + priority-ordered, striped over 3 queues ----
        # sync/gpsimd/vector: wk/wq s0-slices then x column blocks in
        # consumption order; scalar (otherwise idle): wv then wo.
        # Later: out-DMAs ride sync only; normalize DMAs ride gpsimd only.
        # critical prefix (wk/wq s0-slices + first x column block) in ~128KB
        # pieces round-robined over all 3 queues: minimizes the makespan of
        # the slowest queue, which gates the first k/q projections
        prefix = [
            (wk[:, 0:4, 0:128], w_d.ap()[:, 1, 0:4, 0:128]),
            (wq[:, 0:4, 0:128], w_d.ap()[:, 2, 0:4, 0:128]),
            (xc[0][:, 0:2, :], xT_d.ap()[:, 0, 0:2, :]),
            (xc[0][:, 2:4, :], xT_d.ap()[:, 0, 2:4, :]),
            (wk[:, 4:8, 0:128], w_d.ap()[:, 1, 4:8, 0:128]),
            (wq[:, 4:8, 0:128], w_d.ap()[:, 2, 4:8, 0:128]),
            (xc[0][:, 4:6, :], xT_d.ap()[:, 0, 4:6, :]),
            (xc[0][:, 6:8, :], xT_d.ap()[:, 0, 6:8, :]),
        ]
        for i, (dst, srcap) in enumerate(prefix):
            eng = (nc.sync, nc.gpsimd, nc.scalar)[i % 3]
            eng.dma_start(out=dst, in_=srcap)
        for c in range(1, NQB):
            # contiguous thirds of each later 1MB x column block
            for j, (lo, hi) in enumerate(((0, 3), (3, 6), (6, 8))):
                eng = (nc.sync, nc.gpsimd, nc.scalar)[(c + j) % 3]
                eng.dma_start(out=xc[c][:, lo:hi, :],
                              in_=xT_d.ap()[:, c, lo:hi, :])
            if c == 1:
                # wv early-ish (needed by vchunk(0) in block 0)
                nc.scalar.dma_start(out=wv, in_=w_d.ap()[:, 0])
        nc.sync.dma_start(out=wk[:, :, 128:512], in_=w_d.ap()[:, 1, :, 128:512])
        nc.gpsimd.dma_start(out=wq[:, :, 128:512], in_=w_d.ap()[:, 2, :, 128:512])
        nc.scalar.dma_start(out=wo_sb, in_=wo_d.ap())

        nc.vector.memset(vt[:, :, :, DH], 1.0)

        # ---- HAM warm-up: garbage MMs bridge the short pre-data window; the
        # first projections then keep the PE busy and the clock-gate warm.
        ps_w0 = utilp.tile([128, 512], F32, tag="u", name="ps_w0")
        ps_w1 = utilp.tile([128, 512], F32, tag="u", name="ps_w1")
        for i in range(28):
            wb = 64 * (i % 2)
            nc.tensor.matmul(ps_w0 if wb == 0 else ps_w1,
                             kTs[0][wb:wb + 64, 0:128],
                             kTs[0][wb:wb + 64, 0:512],
                             start=True, stop=True, tile_position=(wb, 0))

        # ---- projection emitters (PE work chunks; all write via util pool) ----
        def kq_half(w, dst, s, c, half, cell):
            """4 accumulation MMs; both halves share one PSUM tile via `cell`;
            half 1 finishes the group + copies out."""
            if half == 0:
                cell["ps"] = utilp.tile([128, 512], F32, tag="u",
                                        name=f"kq{s}_{c}_{w is wq}")
            ps = cell["ps"]
            for kt in range(4 * half, 4 * half + 4):
                nc.tensor.matmul(ps, w[:, kt, 128 * s:128 * (s + 1)],
                                 xc[c][:, kt, :],
                                 start=(kt == 0), stop=(kt == KT - 1))
            if half == 1:
                nc.vector.tensor_copy(out=dst[s][:, 512 * c:512 * (c + 1)], in_=ps)

        def kq_chunk(w, dst, s, c):
            cell = {}
            kq_half(w, dst, s, c, 0, cell)
            kq_half(w, dst, s, c, 1, cell)

        def vchunk(t):
            """v projection for token tile t, all 8 heads (512-wide moving)."""
            ps = utilp.tile([128, 512], F32, tag="u", name=f"vps{t}")
            for kt in range(KT):
                nc.tensor.matmul(
                    ps, xc[t // 4][:, kt, 128 * (t % 4):128 * (t % 4 + 1)],
                    wv[:, kt, :],
                    start=(kt == 0), stop=(kt == KT - 1))
            nc.vector.tensor_copy(
                out=vt[:, t, :, 0:DH],
                in_=ps.rearrange("p (h d) -> p h d", h=HPC))

        def oproj_half(ns, c):
            """output projection for token rows 128*ns.., output cols 512*c.."""
            po = utilp.tile([128, 512], F32, tag="u", name=f"po{ns}_{c}")
            for hp in range(NPAIR):
                nc.tensor.matmul(
                    po, aoTs[hp][:, 128 * ns:128 * (ns + 1)],
                    wo_sb[:, hp, 512 * c:512 * (c + 1)],
                    start=(hp == 0), stop=(hp == NPAIR - 1))
            st = stp.tile([128, 512], BF16, tag="st", name=f"st{ns}_{c}")
            nc.vector.tensor_copy(out=st, in_=po)
            nc.sync.dma_start(
                out=out_d.ap()[128 * ns:128 * (ns + 1), 512 * c:512 * (c + 1)],
                in_=st)

        # ---- filler queue: generator items emitting ~460ns sub-chunks ----
        # Items are driven strictly head-first (one active generator at a
        # time), so a multi-sub item's open PSUM accumulation group is never
        # interleaved with another item's util-pool allocation. The queue is
        # pushed in non-decreasing deadline order; oproj items (deadline 98)
        # are appended at the end as their inputs complete.
        fillers = []   # entries: [subcost, deadline, generator]
        reserve = []   # oproj generators held back for the tail bridge
        pending = []   # oproj items awaiting their release block

        def kq_gen(w, dst, s, c):
            ps = utilp.tile([128, 512], F32, tag="u", name=f"kq{s}_{c}_{w is wq}")
            for kt in range(KT):
                nc.tensor.matmul(ps, w[:, kt, 128 * s:128 * (s + 1)],
                                 xc[c][:, kt, :],
                                 start=(kt == 0), stop=(kt == KT - 1))
                if kt % 2 == 1:
                    if kt == KT - 1:
                        nc.vector.tensor_copy(
                            out=dst[s][:, 512 * c:512 * (c + 1)], in_=ps)
                    yield

        def oproj_gen(ns, c, tail=False):
            po = utilp.tile([128, 512], F32, tag="u", name=f"po{ns}_{c}")
            for hp in range(NPAIR):
                nc.tensor.matmul(
                    po, aoTs[hp][:, 128 * ns:128 * (ns + 1)],
                    wo_sb[:, hp, 512 * c:512 * (c + 1)],
                    start=(hp == 0), stop=(hp == NPAIR - 1))
                if hp == 1:
                    yield
            # tail stores ride the (then idle) scalar queue + a deep staging
            # pool so they never contend with the final normalize DMA chains
            pool = stpt if tail else stp
            st = pool.tile([128, 512], BF16, tag="st", name=f"st{ns}_{c}")
            nc.vector.tensor_copy(out=st, in_=po)
            eng = nc.scalar if tail else nc.sync
            eng.dma_start(
                out=out_d.ap()[128 * ns:128 * (ns + 1), 512 * c:512 * (c + 1)],
                in_=st)
            yield

        def push_kq(w, dst, s, c, deadline):
            fillers.append([460, deadline, kq_gen(w, dst, s, c)])

        def emit_one_sub():
            """advance the head generator by one sub-chunk; returns its cost"""
            while fillers:
                item = fillers[0]
                try:
                    next(item[2])
                    return item[0]
                except StopIteration:
                    fillers.pop(0)
            return None

        # block order: s0 pass (blocks 0-3), s1 pass (4-7), then s2/s3
        # interleaved by qb (8-15) so oproj(qb) unblocks early.
        # first-block index per (s, qb):
        def blk_of(s, qb):
            if s < 2:
                return 4 * s + qb
            return 8 + 2 * qb + (s - 2)

        # q chunks c=1..3 for s=0 are needed by blocks 1,2,3
        for c in range(1, NQB):
            push_kq(wq, qTs, 0, c, blk_of(0, c))
        for s in range(1, NPAIR):
            for c in range(NQB):
                push_kq(wk, kTs, s, c, blk_of(s, 0))
            push_kq(wq, qTs, s, 0, blk_of(s, 0))
            for c in range(1, NQB):
                push_kq(wq, qTs, s, c, blk_of(s, c))
        # oproj items are appended dynamically once block (3, qb) completes.

        budget = [0.0]
        STEP_COST = 740.0
        ACT_STEP = 1147.0

        def force_drain(blk_idx):
            # emit everything this block (or earlier) depends on; the queue is
            # deadline-sorted so head-first driving is sufficient
            while fillers and fillers[0][1] <= blk_idx:
                item = fillers[0]
                try:
                    next(item[2])
                    budget[0] -= item[0]
                except StopIteration:
                    fillers.pop(0)

        def pop_budget():
            while fillers and budget[0] >= fillers[0][0]:
                cost = emit_one_sub()
                if cost is None:
                    break
                budget[0] -= cost

        # ---- attention block ----
        def block(s, qb, blk_idx, jit_v=False):
            # fillers this block depends on (k/q chunks) MUST be emitted
            # before the block's first score matmul, or the PE FIFO deadlocks
            budget[0] = max(budget[0], -4000.0)
            force_drain(blk_idx)
            h0, h1 = 2 * s, 2 * s + 1
            av0 = avp.tile([DH + 1, QB], F32, tag="av", name=f"av0_{s}_{qb}")
            av1 = avp.tile([DH + 1, QB], F32, tag="av", name=f"av1_{s}_{qb}")
            pend = []

            def emit_av(t, at):
                nc.tensor.matmul(av0, vt[:, t, h0, :], at[:, 0:QB],
                                 start=(t == 0), stop=(t == NT - 1))
                nc.tensor.matmul(av1, vt[:, t, h1, :], at[:, QB:2 * QB],
                                 start=(t == 0), stop=(t == NT - 1))

            for t in range(NT):
                sc = scp.tile([128, 2 * QB], F32, tag="sc",
                              name=f"sc{s}_{qb}_{t}")
                # both heads' scores back-to-back -> disjoint row groups run
                # concurrently on the PE
                nc.tensor.matmul(sc[:, 0:QB],
                                 kTs[s][0:64, 128 * t:128 * (t + 1)],
                                 qTs[s][0:64, QB * qb:QB * (qb + 1)],
                                 start=True, stop=True, tile_position=(0, 0))
                nc.tensor.matmul(sc[:, QB:2 * QB],
                                 kTs[s][64:128, 128 * t:128 * (t + 1)],
                                 qTs[s][64:128, QB * qb:QB * (qb + 1)],
                                 start=True, stop=True, tile_position=(64, 0))
                at = atp.tile([128, 2 * QB], BF16, tag="at",
                              name=f"at{s}_{qb}_{t}")
                nc.scalar.activation(out=at, in_=sc,
                                     func=mybir.ActivationFunctionType.Exp,
                                     scale=SCALE)
                # fillers slot in here: the PE would otherwise idle waiting
                # for exp(t-2) to finish before the lag-2 AV can consume it
                if not jit_v:
                    budget[0] += ACT_STEP - STEP_COST
                    pop_budget()
                # AV lags two steps: its `at` input was finished by the exp
                # one full step ago, so the PE never waits on the ACT engine
                if len(pend) == 2:
                    emit_av(*pend.pop(0))
                pend.append((t, at))
                if jit_v:
                    # v projection for tile t lands well before its AV
                    vchunk(t)
                    if t % 4 == 3 and t < 12:
                        # next k column block, ahead of its score deadline
                        kq_chunk(wk, kTs, 0, t // 4 + 1)
            while pend:
                emit_av(*pend.pop(0))
            for p in (1, 0):
                normalize(s, qb, p, (av0, av1)[p])

        def normalize(s, qb, p, av):
            sfx = f"{s}_{qb}_{p}"
            # p1 chain rides gpsimd, p0 rides sync: the two per-block chains
            # (3 serial DMA hops each) run in parallel instead of queueing
            dma = nc.gpsimd.dma_start if p == 1 else nc.sync.dma_start
            avu = avup.tile([DH + 1, QB], F32, tag="avu", name=f"avu{sfx}")
            nc.vector.tensor_copy(out=avu, in_=av)
            # spread the denominator over 64 partitions so the reciprocal runs
            # 64-wide instead of 8 cycles/elem on a single partition
            dsp = rcp.tile([DH, QB // DH], F32, tag="dsp", name=f"dsp{sfx}")
            dma(out=dsp, in_=avu[DH:DH + 1, :])
            rc = rcp.tile([DH, QB // DH], F32, tag="rc", name=f"rc{sfx}")
            nc.vector.reciprocal(out=rc, in_=dsp)
            dr = drbp.tile([QB], F32, tag="dr", name=f"dr{sfx}")
            dr_sq = bass.AP(tensor=dr.tensor, offset=dr.offset,
                            ap=[[QB // DH, DH], [1, QB // DH]])
            dma(out=dr_sq, in_=rc)
            dr_bc = bass.AP(tensor=dr.tensor, offset=dr.offset,
                            ap=[[0, DH]] + [list(dd) for dd in dr.ap])
            bcs = bcsp.tile([DH, QB], F32, tag="bcs", name=f"bcs{sfx}")
            dma(out=bcs, in_=dr_bc)
            if p == 0:
                with nc.allow_low_precision(reason="attn out in bf16"):
                    nc.vector.tensor_mul(
                        out=aoTs[s][0:DH, QB * qb:QB * (qb + 1)],
                        in0=avu[0:DH, :], in1=bcs)
            else:
                od = oddp.tile([DH, QB], BF16, tag="od", name=f"od{sfx}")
                with nc.allow_low_precision(reason="attn out in bf16"):
                    nc.vector.tensor_mul(out=od, in0=avu[0:DH, :], in1=bcs)
                nc.gpsimd.dma_start(
                    out=aoTs[s][DH:128, QB * qb:QB * (qb + 1)], in_=od)

        # ---- pre-phase: first k/q chunks for block (0,0) ----
        kq_chunk(wk, kTs, 0, 0)
        kq_chunk(wq, qTs, 0, 0)

        # ---- main loop: s0 pass, s1 pass, then s2/s3 interleaved by qb ----
        order = ([(0, qb) for qb in range(NQB)] +
                 [(1, qb) for qb in range(NQB)] +
                 [(s, qb) for qb in range(NQB) for s in (2, 3)])
        for blk, (s, qb) in enumerate(order):
            while pending and pending[0][0] <= blk - 2:
                src_blk, cost, gen = pending.pop(0)
                fillers.append([cost, 98, gen])
            block(s, qb, blk, jit_v=(blk == 0))
            if s == NPAIR - 1 and qb < NQB - 1:
                # aoT rows for this qb now complete -> queue output proj.
                # qb0/qb1 go to `pending` (released one full block later so
                # their first matmul never waits on this block's od-DMA and
                # stalls the PE FIFO); all of qb2 is reserved as tail bridge
                # work to keep the PE busy+warm across the final normalize.
                for ns in range(4 * qb, 4 * qb + 4):
                    for c in range(2):
                        if qb == NQB - 2:
                            reserve.append(oproj_gen(ns, c, tail=True))
                        else:
                            pending.append([blk, 640, oproj_gen(ns, c)])

        # ---- tail: drain leftover fillers, then the last qb's output
        # projection with its hp=0..2 accumulation prefix emitted DURING the
        # final normalize (only the hp=3 matmul + cast + store depend on it).
        # The prefix MMs double as HAM warm-guards across the normalize wait.
        for src_blk, cost, gen in pending:
            fillers.append([cost, 98, gen])
        pending.clear()
        while emit_one_sub() is not None:
            pass
        for gen in reserve:
            for _ in gen:
                pass
        # short garbage burst: keeps the PE busy (and the clock-gate warm)
        # through the remainder of the final normalize's DMA chain
        for i in range(18):
            wb = 64 * (i % 2)
            ps_wg = utilp.tile([128, 512], F32, tag="u", name=f"twg{i}")
            nc.tensor.matmul(ps_wg, kTs[0][wb:wb + 64, 0:128],
                             kTs[0][wb:wb + 64, 0:512],
                             start=True, stop=True, tile_position=(wb, 0))
        tail_halves = [(ns, c) for ns in range(4 * (NQB - 1), 4 * NQB)
                       for c in range(2)]
        slots = []
        for i in range(2):
            sct = scp.tile([128, 2 * QB], F32, tag="sc", name=f"tpo{i}")
            slots += [sct[:, 0:512], sct[:, 512:1024]]
        for i in range(2):
            slots.append(utilp.tile([128, 512], F32, tag="u", name=f"tpo_u{i}"))
        for i in range(2):
            slots.append(avp.tile([128, 512], F32, tag="av", name=f"tpo_a{i}"))
        for idx, (ns, c) in enumerate(tail_halves):
            for hp in range(NPAIR - 1):
                nc.tensor.matmul(
                    slots[idx], aoTs[hp][:, 128 * ns:128 * (ns + 1)],
                    wo_sb[:, hp, 512 * c:512 * (c + 1)],
                    start=(hp == 0), stop=False)
        for idx, (ns, c) in enumerate(tail_halves):
            nc.tensor.matmul(
                slots[idx], aoTs[NPAIR - 1][:, 128 * ns:128 * (ns + 1)],
                wo_sb[:, NPAIR - 1, 512 * c:512 * (c + 1)],
                start=False, stop=True)
            st = stpt.tile([128, 512], BF16, tag="st", name=f"tst{ns}_{c}")
            nc.vector.tensor_copy(out=st, in_=slots[idx])
            eng = (nc.scalar, nc.sync)[idx % 2]
            eng.dma_start(
                out=out_d.ap()[128 * ns:128 * (ns + 1), 512 * c:512 * (c + 1)],
                in_=st)


def get_program():
    if "nc" not in _CACHE:
        _CACHE["nc"] = _build_program()
    return _CACHE["nc"]


def make_in_maps(x, w_qkv, w_out, b_out):
    bf = ml_dtypes.bfloat16
    x = np.asarray(x, np.float32)
    w_qkv = np.asarray(w_qkv, np.float32)
    w_out = np.asarray(w_out, np.float32)
    b_out = np.asarray(b_out, np.float32)

    in_maps = []
    for core in range(NCORES):
        b, hh = core // 2, core % 2
        # xT in [128, NQB, KT, 512] layout: [p, c, t, e] = x[b].T[128t+p, 512c+e]
        # (column blocks contiguous so each input DMA is a dense 3KB+/partition
        # transfer instead of strided 1KB rows)
        xT = np.ascontiguousarray(x[b].T).astype(bf)                 # [DIM, N]
        xT_pt = np.ascontiguousarray(
            xT.reshape(KT, 128, NQB, 512).transpose(1, 2, 0, 3))
        # w slices for this head-half, groups ordered [v, k, q]
        wq = w_qkv[:, 512 * hh:512 * (hh + 1)]
        wk = w_qkv[:, DIM + 512 * hh:DIM + 512 * (hh + 1)]
        wv = w_qkv[:, 2 * DIM + 512 * hh:2 * DIM + 512 * (hh + 1)]
        wcat = np.stack([wv, wk, wq], axis=0).astype(bf)             # [3, DIM, 512]
        w_pt = np.ascontiguousarray(
            wcat.reshape(3, KT, 128, 512).transpose(2, 0, 1, 3))    # [p, g, t, e]
        # w_out rows for this half -> [p, hp, d]
        wo = w_out[512 * hh:512 * (hh + 1), :].astype(bf)            # [512, DIM]
        wo_pt = np.ascontiguousarray(wo.reshape(NPAIR, 128, DIM).transpose(1, 0, 2))
        in_maps.append({
            "xT": xT_pt,
            "w_qkv": w_pt,
            "w_out": wo_pt,
        })
    return in_maps


def kernel(x, w_qkv, w_out, b_out):
    nc = get_program()
    in_maps = make_in_maps(x, w_qkv, w_out, b_out)
    res = bass_utils.run_bass_kernel_spmd(nc, in_maps, core_ids=list(range(NCORES)))
    out = np.empty((B, N, DIM), np.float32)
    bias = np.asarray(b_out, np.float32)
    for b in range(B):
        out[b] = np.asarray(res.results[2 * b]["out"], np.float32)
        out[b] += np.asarray(res.results[2 * b + 1]["out"], np.float32)
        out[b] += bias
    return out


# revision 30
# speedup vs baseline: 1.0272x; 1.0272x over previous
"""Trainium2 Bass kernel for multi-head attention (B=4, N=2048, DIM=1024, H=16, DH=64).

Sharding (head-parallel + row-parallel to_out): 8 cores = 4 batches x 2 head-halves.
Each core computes q/k/v for its 8 heads over the FULL 2048-token sequence, runs
attention for those heads, and its row-parallel half of the output projection. The
to_out all-reduce happens on the host at gather time:
out[b] = partial[core 2b] + partial[core 2b+1] + bias.

The attention phase is ACT(exp)-bound: softmax exp runs only on the scalar engine at
1 elem/cycle/lane, so the per-core floor is 256 x [128,1024]-exp = ~294 us. This
kernel keeps the ACT stream contiguous from ~18us on by:
  * blocks of (head-pair s, 512-query block qb): both heads' scores live side by
    side in ONE [128,1024] fp32 PSUM tile (2 banks) -> one exp instruction per
    key-tile step covers both heads at full FD=1024 efficiency.
  * score matmuls for the two heads are issued back-to-back into disjoint PE row
    groups (tile_position (0,0)/(64,0)) so they stream concurrently (~2x).
  * av accumulators are [65,512] (1 PSUM bank each, ones-column denominator), so
    PSUM = 4(sc double-buffer) + 2(av) + 2(util) banks, leaving a util pool for
    projection matmuls to interleave with attention.
  * input DMAs are chunked and priority-ordered (first key/query weight slices,
    x column blocks in consumption order), so the first exp fires at ~18us; the
    qkv projections, late q chunks, and the output projection stream into the
    per-step PE slack via a budget-tracked filler queue.
  * a ~44-matmul garbage warm-up burst keeps the PE HAM clock-gate busy across
    the initial DMA wait so projection matmuls run at 2.4 GHz from the start.
Softmax denominators are folded into the AV matmul via a ones-column in V; the
reciprocal is spread over 64 partitions via an SBUF DMA restructure, then
broadcast back through a DRAM round-trip.
"""

import numpy as np
import ml_dtypes

import concourse.bass as bass
import concourse.tile as tile
from concourse import bacc, mybir
from concourse import bass_utils

B, N, DIM = 4, 2048, 1024
HEADS_TOT, DH = 16, 64
SCALE = DH ** -0.5
NCORES = 8

HPC = 8              # heads per core
NPAIR = HPC // 2     # head-pairs per core = 4
KT = DIM // 128      # 8 contraction tiles
NT = N // 128        # 16 key tiles
QB = 512             # queries per block
NQB = N // QB        # 4 query blocks
BF16 = mybir.dt.bfloat16
F32 = mybir.dt.float32

_CACHE = {}


def _build_program():
    nc = bacc.Bacc("TRN2", target_bir_lowering=False, debug=False)

    xT_d = nc.dram_tensor("xT", [128, NQB, KT, 512], BF16, kind="ExternalInput")
    w_d = nc.dram_tensor("w_qkv", [128, 3, KT, 512], BF16, kind="ExternalInput")
    wo_d = nc.dram_tensor("w_out", [128, NPAIR, DIM], BF16, kind="ExternalInput")
    out_d = nc.dram_tensor("out", [N, DIM], BF16, kind="ExternalOutput")

    with tile.TileContext(nc) as tc:
        _emit(tc, nc, xT_d, w_d, wo_d, out_d)
    nc.compile()
    return nc


def _emit(tc, nc, xT_d, w_d, wo_d, out_d):
    from contextlib import ExitStack

    with ExitStack() as ctx:
        consts = ctx.enter_context(tc.tile_pool(name="consts", bufs=1))
        stage = ctx.enter_context(tc.tile_pool(name="stage", bufs=1))
        qkv = ctx.enter_context(tc.tile_pool(name="qkv", bufs=1))
        ao = ctx.enter_context(tc.tile_pool(name="ao", bufs=1))
        atp = ctx.enter_context(tc.tile_pool(name="atp", bufs=6))
        avup = ctx.enter_context(tc.tile_pool(name="avu", bufs=2))
        rcp = ctx.enter_context(tc.tile_pool(name="rcp", bufs=2))
        bcsp = ctx.enter_context(tc.tile_pool(name="bcs", bufs=2))
        oddp = ctx.enter_context(tc.tile_pool(name="odd", bufs=2))
        drbp = ctx.enter_context(tc.tile_pool(name="drb", bufs=2, space="DRAM"))
        stp = ctx.enter_context(tc.tile_pool(name="stp", bufs=2))
        stpt = ctx.enter_context(tc.tile_pool(name="stt", bufs=8))
        scp = ctx.enter_context(tc.tile_pool(name="scp", bufs=2, space="PSUM"))
        avp = ctx.enter_context(tc.tile_pool(name="avp", bufs=2, space="PSUM"))
        utilp = ctx.enter_context(tc.tile_pool(name="utl", bufs=2, space="PSUM"))

        # ---- persistent SBUF tiles ----
        wo_sb = consts.tile([128, NPAIR, DIM], BF16)
        at_warm = consts.tile([128, 32], BF16)
        wv = stage.tile([128, KT, 512], BF16)
        wk = stage.tile([128, KT, 512], BF16)
        wq = stage.tile([128, KT, 512], BF16)
        xc = [stage.tile([128, KT, 512], BF16, name=f"xc{c}") for c in range(NQB)]
        kTs = [qkv.tile([128, N], BF16, name=f"kT{s}") for s in range(NPAIR)]
        qTs = [qkv.tile([128, N], BF16, name=f"qT{s}") for s in range(NPAIR)]
        vt = qkv.tile([128, NT, HPC, DH + 1], BF16)
        aoTs = [ao.tile([128, N], BF16, name=f"aoT{s}") for s in range(NPAIR)]

        # ---- exp table preload (reads garbage SBUF; off the data critical path)
        nc.scalar.activation(out=at_warm, in_=kTs[0][:, 0:32],
                             func=mybir.ActivationFunctionType.Exp)

        # ---- input DMAs: chunked + priority-ordered, striped over 3 queues ----
        # sync/gpsimd/vector: wk/wq s0-slices then x column blocks in
        # consumption order; scalar (otherwise idle): wv then wo.
        # Later: out-DMAs ride sync only; normalize DMAs ride gpsimd only.
        nc.sync.dma_start(out=wk[:, :, 0:128], in_=w_d.ap()[:, 1, :, 0:128])
        nc.gpsimd.dma_start(out=wq[:, :, 0:128], in_=w_d.ap()[:, 2, :, 0:128])
        for c in range(NQB):
            # contiguous thirds of each 1MB x column block, striped 3 ways
            for j, (lo, hi) in enumerate(((0, 3), (3, 6), (6, 8))):
                eng = (nc.sync, nc.gpsimd, nc.scalar)[(c + j) % 3]
                eng.dma_start(out=xc[c][:, lo:hi, :],
                              in_=xT_d.ap()[:, c, lo:hi, :])
            if c == 0:
                # wv after the first x column block (needed by vchunk(0) only)
                nc.scalar.dma_start(out=wv, in_=w_d.ap()[:, 0])
        nc.sync.dma_start(out=wk[:, :, 128:512], in_=w_d.ap()[:, 1, :, 128:512])
        nc.gpsimd.dma_start(out=wq[:, :, 128:512], in_=w_d.ap()[:, 2, :, 128:512])
        nc.scalar.dma_start(out=wo_sb, in_=wo_d.ap())

        nc.vector.memset(vt[:, :, :, DH], 1.0)

        # ---- HAM warm-up: garbage MMs bridge the short pre-data window; the
        # first projections then keep the PE busy and the clock-gate warm.
        ps_w0 = utilp.tile([128, 512], F32, tag="u", name="ps_w0")
        ps_w1 = utilp.tile([128, 512], F32, tag="u", name="ps_w1")
        for i in range(28):
            wb = 64 * (i % 2)
            nc.tensor.matmul(ps_w0 if wb == 0 else ps_w1,
                             kTs[0][wb:wb + 64, 0:128],
                             kTs[0][wb:wb + 64, 0:512],
                             start=True, stop=True, tile_position=(wb, 0))

        # ---- projection emitters (PE work chunks; all write via util pool) ----
        def kq_half(w, dst, s, c, half, cell):
            """4 accumulation MMs; both halves share one PSUM tile via `cell`;
            half 1 finishes the group + copies out."""
            if half == 0:
                cell["ps"] = utilp.tile([128, 512], F32, tag="u",
                                        name=f"kq{s}_{c}_{w is wq}")
            ps = cell["ps"]
            for kt in range(4 * half, 4 * half + 4):
                nc.tensor.matmul(ps, w[:, kt, 128 * s:128 * (s + 1)],
                                 xc[c][:, kt, :],
                                 start=(kt == 0), stop=(kt == KT - 1))
            if half == 1:
                nc.vector.tensor_copy(out=dst[s][:, 512 * c:512 * (c + 1)], in_=ps)

        def kq_chunk(w, dst, s, c):
            cell = {}
            kq_half(w, dst, s, c, 0, cell)
            kq_half(w, dst, s, c, 1, cell)

        def vchunk(t):
            """v projection for token tile t, all 8 heads (512-wide moving)."""
            ps = utilp.tile([128, 512], F32, tag="u", name=f"vps{t}")
            for kt in range(KT):
                nc.tensor.matmul(
                    ps, xc[t // 4][:, kt, 128 * (t % 4):128 * (t % 4 + 1)],
                    wv[:, kt, :],
                    start=(kt == 0), stop=(kt == KT - 1))
            nc.vector.tensor_copy(
                out=vt[:, t, :, 0:DH],
                in_=ps.rearrange("p (h d) -> p h d", h=HPC))

        def oproj_half(ns, c):
            """output projection for token rows 128*ns.., output cols 512*c.."""
            po = utilp.tile([128, 512], F32, tag="u", name=f"po{ns}_{c}")
            for hp in range(NPAIR):
                nc.tensor.matmul(
                    po, aoTs[hp][:, 128 * ns:128 * (ns + 1)],
                    wo_sb[:, hp, 512 * c:512 * (c + 1)],
                    start=(hp == 0), stop=(hp == NPAIR - 1))
            st = stp.tile([128, 512], BF16, tag="st", name=f"st{ns}_{c}")
            nc.vector.tensor_copy(out=st, in_=po)
            nc.sync.dma_start(
                out=out_d.ap()[128 * ns:128 * (ns + 1), 512 * c:512 * (c + 1)],
                in_=st)

        # ---- filler queue: generator items emitting ~460ns sub-chunks ----
        # Items are driven strictly head-first (one active generator at a
        # time), so a multi-sub item's open PSUM accumulation group is never
        # interleaved with another item's util-pool allocation. The queue is
        # pushed in non-decreasing deadline order; oproj items (deadline 98)
        # are appended at the end as their inputs complete.
        fillers = []   # entries: [subcost, deadline, generator]
        reserve = []   # oproj generators held back for the tail bridge
        pending = []   # oproj items awaiting their release block

        def kq_gen(w, dst, s, c):
            ps = utilp.tile([128, 512], F32, tag="u", name=f"kq{s}_{c}_{w is wq}")
            for kt in range(KT):
                nc.tensor.matmul(ps, w[:, kt, 128 * s:128 * (s + 1)],
                                 xc[c][:, kt, :],
                                 start=(kt == 0), stop=(kt == KT - 1))
                if kt % 2 == 1:
                    if kt == KT - 1:
                        nc.vector.tensor_copy(
                            out=dst[s][:, 512 * c:512 * (c + 1)], in_=ps)
                    yield

        def oproj_gen(ns, c, tail=False):
            po = utilp.tile([128, 512], F32, tag="u", name=f"po{ns}_{c}")
            for hp in range(NPAIR):
                nc.tensor.matmul(
                    po, aoTs[hp][:, 128 * ns:128 * (ns + 1)],
                    wo_sb[:, hp, 512 * c:512 * (c + 1)],
                    start=(hp == 0), stop=(hp == NPAIR - 1))
                if hp == 1:
                    yield
            # tail stores ride the (then idle) scalar queue + a deep staging
            # pool so they never contend with the final normalize DMA chains
            pool = stpt if tail else stp
            st = pool.tile([128, 512], BF16, tag="st", name=f"st{ns}_{c}")
            nc.vector.tensor_copy(out=st, in_=po)
            eng = nc.scalar if tail else nc.sync
            eng.dma_start(
                out=out_d.ap()[128 * ns:128 * (ns + 1), 512 * c:512 * (c + 1)],
                in_=st)
            yield

        def push_kq(w, dst, s, c, deadline):
            fillers.append([460, deadline, kq_gen(w, dst, s, c)])

        def emit_one_sub():
            """advance the head generator by one sub-chunk; returns its cost"""
            while fillers:
                item = fillers[0]
                try:
                    next(item[2])
                    return item[0]
                except StopIteration:
                    fillers.pop(0)
            return None

        # block order: s0 pass (blocks 0-3), s1 pass (4-7), then s2/s3
        # interleaved by qb (8-15) so oproj(qb) unblocks early.
        # first-block index per (s, qb):
        def blk_of(s, qb):
            if s < 2:
                return 4 * s + qb
            return 8 + 2 * qb + (s - 2)

        # q chunks c=1..3 for s=0 are needed by blocks 1,2,3
        for c in range(1, NQB):
            push_kq(wq, qTs, 0, c, blk_of(0, c))
        for s in range(1, NPAIR):
            for c in range(NQB):
                push_kq(wk, kTs, s, c, blk_of(s, 0))
            push_kq(wq, qTs, s, 0, blk_of(s, 0))
            for c in range(1, NQB):
                push_kq(wq, qTs, s, c, blk_of(s, c))
        # oproj items are appended dynamically once block (3, qb) completes.

        budget = [0.0]
        STEP_COST = 740.0
        ACT_STEP = 1147.0

        def force_drain(blk_idx):
            # emit everything this block (or earlier) depends on; the queue is
            # deadline-sorted so head-first driving is sufficient
            while fillers and fillers[0][1] <= blk_idx:
                item = fillers[0]
                try:
                    next(item[2])
                    budget[0] -= item[0]
                except StopIteration:
                    fillers.pop(0)

        def pop_budget():
            while fillers and budget[0] >= fillers[0][0]:
                cost = emit_one_sub()
                if cost is None:
                    break
                budget[0] -= cost

        # ---- attention block ----
        def block(s, qb, blk_idx, jit_v=False):
            # fillers this block depends on (k/q chunks) MUST be emitted
            # before the block's first score matmul, or the PE FIFO deadlocks
            budget[0] = max(budget[0], -4000.0)
            force_drain(blk_idx)
            h0, h1 = 2 * s, 2 * s + 1
            av0 = avp.tile([DH + 1, QB], F32, tag="av", name=f"av0_{s}_{qb}")
            av1 = avp.tile([DH + 1, QB], F32, tag="av", name=f"av1_{s}_{qb}")
            pend = []

            def emit_av(t, at):
                nc.tensor.matmul(av0, vt[:, t, h0, :], at[:, 0:QB],
                                 start=(t == 0), stop=(t == NT - 1))
                nc.tensor.matmul(av1, vt[:, t, h1, :], at[:, QB:2 * QB],
                                 start=(t == 0), stop=(t == NT - 1))

            for t in range(NT):
                sc = scp.tile([128, 2 * QB], F32, tag="sc",
                              name=f"sc{s}_{qb}_{t}")
                # both heads' scores back-to-back -> disjoint row groups run
                # concurrently on the PE
                nc.tensor.matmul(sc[:, 0:QB],
                                 kTs[s][0:64, 128 * t:128 * (t + 1)],
                                 qTs[s][0:64, QB * qb:QB * (qb + 1)],
                                 start=True, stop=True, tile_position=(0, 0))
                nc.tensor.matmul(sc[:, QB:2 * QB],
                                 kTs[s][64:128, 128 * t:128 * (t + 1)],
                                 qTs[s][64:128, QB * qb:QB * (qb + 1)],
                                 start=True, stop=True, tile_position=(64, 0))
                at = atp.tile([128, 2 * QB], BF16, tag="at",
                              name=f"at{s}_{qb}_{t}")
                nc.scalar.activation(out=at, in_=sc,
                                     func=mybir.ActivationFunctionType.Exp,
                                     scale=SCALE)
                # fillers slot in here: the PE would otherwise idle waiting
                # for exp(t-2) to finish before the lag-2 AV can consume it
                if not jit_v:
                    budget[0] += ACT_STEP - STEP_COST
                    pop_budget()
                # AV lags two steps: its `at` input was finished by the exp
                # one full step ago, so the PE never waits on the ACT engine
                if len(pend) == 2:
                    emit_av(*pend.pop(0))
                pend.append((t, at))
                if jit_v:
                    # v projection for tile t lands well before its AV
                    vchunk(t)
                    if t % 4 == 3 and t < 12:
                        # next k column block, ahead of its score deadline
                        kq_chunk(wk, kTs, 0, t // 4 + 1)
            while pend:
                emit_av(*pend.pop(0))
            for p in (1, 0):
                normalize(s, qb, p, (av0, av1)[p])

        def normalize(s, qb, p, av):
            sfx = f"{s}_{qb}_{p}"
            # p1 chain rides gpsimd, p0 rides sync: the two per-block chains
            # (3 serial DMA hops each) run in parallel instead of queueing
            dma = nc.gpsimd.dma_start if p == 1 else nc.sync.dma_start
            avu = avup.tile([DH + 1, QB], F32, tag="avu", name=f"avu{sfx}")
            nc.vector.tensor_copy(out=avu, in_=av)
            # spread the denominator over 64 partitions so the reciprocal runs
            # 64-wide instead of 8 cycles/elem on a single partition
            dsp = rcp.tile([DH, QB // DH], F32, tag="dsp", name=f"dsp{sfx}")
            dma(out=dsp, in_=avu[DH:DH + 1, :])
            rc = rcp.tile([DH, QB // DH], F32, tag="rc", name=f"rc{sfx}")
            nc.vector.reciprocal(out=rc, in_=dsp)
            dr = drbp.tile([QB], F32, tag="dr", name=f"dr{sfx}")
            dr_sq = bass.AP(tensor=dr.tensor, offset=dr.offset,
                            ap=[[QB // DH, DH], [1, QB // DH]])
            dma(out=dr_sq, in_=rc)
            dr_bc = bass.AP(tensor=dr.tensor, offset=dr.offset,
                            ap=[[0, DH]] + [list(dd) for dd in dr.ap])
            bcs = bcsp.tile([DH, QB], F32, tag="bcs", name=f"bcs{sfx}")
            dma(out=bcs, in_=dr_bc)
            if p == 0:
                with nc.allow_low_precision(reason="attn out in bf16"):
                    nc.vector.tensor_mul(
                        out=aoTs[s][0:DH, QB * qb:QB * (qb + 1)],
                        in0=avu[0:DH, :], in1=bcs)
            else:
                od = oddp.tile([DH, QB], BF16, tag="od", name=f"od{sfx}")
                with nc.allow_low_precision(reason="attn out in bf16"):
                    nc.vector.tensor_mul(out=od, in0=avu[0:DH, :], in1=bcs)
                nc.gpsimd.dma_start(
                    out=aoTs[s][DH:128, QB * qb:QB * (qb + 1)], in_=od)

        # ---- pre-phase: first k/q chunks for block (0,0) ----
        kq_chunk(wk, kTs, 0, 0)
        kq_chunk(wq, qTs, 0, 0)

        # ---- main loop: s0 pass, s1 pass, then s2/s3 interleaved by qb ----
        order = ([(0, qb) for qb in range(NQB)] +
                 [(1, qb) for qb in range(NQB)] +
                 [(s, qb) for qb in range(NQB) for s in (2, 3)])
        for blk, (s, qb) in enumerate(order):
            while pending and pending[0][0] <= blk - 2:
                src_blk, cost, gen = pending.pop(0)
                fillers.append([cost, 98, gen])
            block(s, qb, blk, jit_v=(blk == 0))
            if s == NPAIR - 1 and qb < NQB - 1:
                # aoT rows for this qb now complete -> queue output proj.
                # qb0/qb1 go to `pending` (released one full block later so
                # their first matmul never waits on this block's od-DMA and
                # stalls the PE FIFO); all of qb2 is reserved as tail bridge
                # work to keep the PE busy+warm across the final normalize.
                for ns in range(4 * qb, 4 * qb + 4):
                    for c in range(2):
                        if qb == NQB - 2:
                            reserve.append(oproj_gen(ns, c, tail=True))
                        else:
                            pending.append([blk, 640, oproj_gen(ns, c)])

        # ---- tail: drain leftover fillers, then the last qb's output
        # projection with its hp=0..2 accumulation prefix emitted DURING the
        # final normalize (only the hp=3 matmul + cast + store depend on it).
        # The prefix MMs double as HAM warm-guards across the normalize wait.
        for src_blk, cost, gen in pending:
            fillers.append([cost, 98, gen])
        pending.clear()
        while emit_one_sub() is not None:
            pass
        for gen in reserve:
            for _ in gen:
                pass
        # short garbage burst: keeps the PE busy (and the clock-gate warm)
        # through the remainder of the final normalize's DMA chain
        for i in range(18):
            wb = 64 * (i % 2)
            ps_wg = utilp.tile([128, 512], F32, tag="u", name=f"twg{i}")
            nc.tensor.matmul(ps_wg, kTs[0][wb:wb + 64, 0:128],
                             kTs[0][wb:wb + 64, 0:512],
                             start=True, stop=True, tile_position=(wb, 0))
        tail_halves = [(ns, c) for ns in range(4 * (NQB - 1), 4 * NQB)
                       for c in range(2)]
        slots = []
        for i in range(2):
            sct = scp.tile([128, 2 * QB], F32, tag="sc", name=f"tpo{i}")
            slots += [sct[:, 0:512], sct[:, 512:1024]]
        for i in range(2):
            slots.append(utilp.tile([128, 512], F32, tag="u", name=f"tpo_u{i}"))
        for i in range(2):
            slots.append(avp.tile([128, 512], F32, tag="av", name=f"tpo_a{i}"))
        for idx, (ns, c) in enumerate(tail_halves):
            for hp in range(NPAIR - 1):
                nc.tensor.matmul(
                    slots[idx], aoTs[hp][:, 128 * ns:128 * (ns + 1)],
                    wo_sb[:, hp, 512 * c:512 * (c + 1)],
                    start=(hp == 0), stop=False)
        for idx, (ns, c) in enumerate(tail_halves):
            nc.tensor.matmul(
                slots[idx], aoTs[NPAIR - 1][:, 128 * ns:128 * (ns + 1)],
                wo_sb[:, NPAIR - 1, 512 * c:512 * (c + 1)],
                start=False, stop=True)
            st = stpt.tile([128, 512], BF16, tag="st", name=f"tst{ns}_{c}")
            nc.vector.tensor_copy(out=st, in_=slots[idx])
            eng = (nc.scalar, nc.sync)[idx % 2]
            eng.dma_start(
                out=out_d.ap()[128 * ns:128 * (ns + 1), 512 * c:512 * (c + 1)],
                in_=st)


def get_program():
    if "nc" not in _CACHE:
        _CACHE["nc"] = _build_program()
    return _CACHE["nc"]


def make_in_maps(x, w_qkv, w_out, b_out):
    bf = ml_dtypes.bfloat16
    x = np.asarray(x, np.float32)
    w_qkv = np.asarray(w_qkv, np.float32)
    w_out = np.asarray(w_out, np.float32)
    b_out = np.asarray(b_out, np.float32)

    in_maps = []
    for core in range(NCORES):
        b, hh = core // 2, core % 2
        # xT in [128, NQB, KT, 512] layout: [p, c, t, e] = x[b].T[128t+p, 512c+e]
        # (column blocks contiguous so each input DMA is a dense 3KB+/partition
        # transfer instead of strided 1KB rows)
        xT = np.ascontiguousarray(x[b].T).astype(bf)                 # [DIM, N]
        xT_pt = np.ascontiguousarray(
            xT.reshape(KT, 128, NQB, 512).transpose(1, 2, 0, 3))
        # w slices for this head-half, groups ordered [v, k, q]
        wq = w_qkv[:, 512 * hh:512 * (hh + 1)]
        wk = w_qkv[:, DIM + 512 * hh:DIM + 512 * (hh + 1)]
        wv = w_qkv[:, 2 * DIM + 512 * hh:2 * DIM + 512 * (hh + 1)]
        wcat = np.stack([wv, wk, wq], axis=0).astype(bf)             # [3, DIM, 512]
        w_pt = np.ascontiguousarray(
            wcat.reshape(3, KT, 128, 512).transpose(2, 0, 1, 3))    # [p, g, t, e]
        # w_out rows for this half -> [p, hp, d]
        wo = w_out[512 * hh:512 * (hh + 1), :].astype(bf)            # [512, DIM]
        wo_pt = np.ascontiguousarray(wo.reshape(NPAIR, 128, DIM).transpose(1, 0, 2))
        in_maps.append({
            "xT": xT_pt,
            "w_qkv": w_pt,
            "w_out": wo_pt,
        })
    return in_maps


def kernel(x, w_qkv, w_out, b_out):
    nc = get_program()
    in_maps = make_in_maps(x, w_qkv, w_out, b_out)
    res = bass_utils.run_bass_kernel_spmd(nc, in_maps, core_ids=list(range(NCORES)))
    out = np.empty((B, N, DIM), np.float32)
    bias = np.asarray(b_out, np.float32)
    for b in range(B):
        out[b] = np.asarray(res.results[2 * b]["out"], np.float32)
        out[b] += np.asarray(res.results[2 * b + 1]["out"], np.float32)
        out[b] += bias
    return out


# revision 34
# speedup vs baseline: 1.0397x; 1.0122x over previous
"""Trainium2 Bass kernel for multi-head attention (B=4, N=2048, DIM=1024, H=16, DH=64).

Sharding (head-parallel + row-parallel to_out): 8 cores = 4 batches x 2 head-halves.
Each core computes q/k/v for its 8 heads over the FULL 2048-token sequence, runs
attention for those heads, and its row-parallel half of the output projection. The
to_out all-reduce happens on the host at gather time:
out[b] = partial[core 2b] + partial[core 2b+1] + bias.

The attention phase is ACT(exp)-bound: softmax exp runs only on the scalar engine at
1 elem/cycle/lane, so the per-core floor is 256 x [128,1024]-exp = ~294 us. This
kernel keeps the ACT stream contiguous from ~18us on by:
  * blocks of (head-pair s, 512-query block qb): both heads' scores live side by
    side in ONE [128,1024] fp32 PSUM tile (2 banks) -> one exp instruction per
    key-tile step covers both heads at full FD=1024 efficiency.
  * score matmuls for the two heads are issued back-to-back into disjoint PE row
    groups (tile_position (0,0)/(64,0)) so they stream concurrently (~2x).
  * av accumulators are [65,512] (1 PSUM bank each, ones-column denominator), so
    PSUM = 4(sc double-buffer) + 2(av) + 2(util) banks, leaving a util pool for
    projection matmuls to interleave with attention.
  * input DMAs are chunked and priority-ordered (first key/query weight slices,
    x column blocks in consumption order), so the first exp fires at ~18us; the
    qkv projections, late q chunks, and the output projection stream into the
    per-step PE slack via a budget-tracked filler queue.
  * a ~44-matmul garbage warm-up burst keeps the PE HAM clock-gate busy across
    the initial DMA wait so projection matmuls run at 2.4 GHz from the start.
Softmax denominators are folded into the AV matmul via a ones-column in V; the
reciprocal is spread over 64 partitions via an SBUF DMA restructure, then
broadcast back through a DRAM round-trip.
"""

import numpy as np
import ml_dtypes

import concourse.bass as bass
import concourse.tile as tile
from concourse import bacc, mybir
from concourse import bass_utils

B, N, DIM = 4, 2048, 1024
HEADS_TOT, DH = 16, 64
SCALE = DH ** -0.5
NCORES = 8

HPC = 8              # heads per core
NPAIR = HPC // 2     # head-pairs per core = 4
KT = DIM // 128      # 8 contraction tiles
NT = N // 128        # 16 key tiles
QB = 512             # queries per block
NQB = N // QB        # 4 query blocks
BF16 = mybir.dt.bfloat16
F32 = mybir.dt.float32

_CACHE = {}


def _build_program():
    nc = bacc.Bacc("TRN2", target_bir_lowering=False, debug=False)

    xT_d = nc.dram_tensor("xT", [128, NQB, KT, 512], BF16, kind="ExternalInput")
    w_d = nc.dram_tensor("w_qkv", [128, 3, KT, 512], BF16, kind="ExternalInput")
    wo_d = nc.dram_tensor("w_out", [128, NPAIR, DIM], BF16, kind="ExternalInput")
    out_d = nc.dram_tensor("out", [N, DIM], BF16, kind="ExternalOutput")

    with tile.TileContext(nc) as tc:
        _emit(tc, nc, xT_d, w_d, wo_d, out_d)
    nc.compile()
    return nc


def _emit(tc, nc, xT_d, w_d, wo_d, out_d):
    from contextlib import ExitStack

    with ExitStack() as ctx:
        consts = ctx.enter_context(tc.tile_pool(name="consts", bufs=1))
        stage = ctx.enter_context(tc.tile_pool(name="stage", bufs=1))
        qkv = ctx.enter_context(tc.tile_pool(name="qkv", bufs=1))
        ao = ctx.enter_context(tc.tile_pool(name="ao", bufs=1))
        atp = ctx.enter_context(tc.tile_pool(name="atp", bufs=6))
        avup = ctx.enter_context(tc.tile_pool(name="avu", bufs=2))
        rcp = ctx.enter_context(tc.tile_pool(name="rcp", bufs=2))
        bcsp = ctx.enter_context(tc.tile_pool(name="bcs", bufs=2))
        oddp = ctx.enter_context(tc.tile_pool(name="odd", bufs=2))
        drbp = ctx.enter_context(tc.tile_pool(name="drb", bufs=2, space="DRAM"))
        stp = ctx.enter_context(tc.tile_pool(name="stp", bufs=2))
        stpt = ctx.enter_context(tc.tile_pool(name="stt", bufs=8))
        scp = ctx.enter_context(tc.tile_pool(name="scp", bufs=2, space="PSUM"))
        avp = ctx.enter_context(tc.tile_pool(name="avp", bufs=2, space="PSUM"))
        utilp = ctx.enter_context(tc.tile_pool(name="utl", bufs=2, space="PSUM"))

        # ---- persistent SBUF tiles ----
        wo_sb = consts.tile([128, NPAIR, DIM], BF16)
        at_warm = consts.tile([128, 32], BF16)
        wv = stage.tile([128, KT, 512], BF16)
        wk = stage.tile([128, KT, 512], BF16)
        wq = stage.tile([128, KT, 512], BF16)
        xc = [stage.tile([128, KT, 512], BF16, name=f"xc{c}") for c in range(NQB)]
        kTs = [qkv.tile([128, N], BF16, name=f"kT{s}") for s in range(NPAIR)]
        qTs = [qkv.tile([128, N], BF16, name=f"qT{s}") for s in range(NPAIR)]
        vt = qkv.tile([128, NT, HPC, DH + 1], BF16)
        aoTs = [ao.tile([128, N], BF16, name=f"aoT{s}") for s in range(NPAIR)]

        # ---- exp table preload (reads garbage SBUF; off the data critical path)
        nc.scalar.activation(out=at_warm, in_=kTs[0][:, 0:32],
                             func=mybir.ActivationFunctionType.Exp)

        # ---- input DMAs: chunked + priority-ordered, striped over 3 queues ----
        # sync/gpsimd/vector: wk/wq s0-slices then x column blocks in
        # consumption order; scalar (otherwise idle): wv then wo.
        # Later: out-DMAs ride sync only; normalize DMAs ride gpsimd only.
        nc.sync.dma_start(out=wk[:, :, 0:128], in_=w_d.ap()[:, 1, :, 0:128])
        nc.gpsimd.dma_start(out=wq[:, :, 0:128], in_=w_d.ap()[:, 2, :, 0:128])
        for c in range(NQB):
            # contiguous thirds of each 1MB x column block, striped 3 ways
            for j, (lo, hi) in enumerate(((0, 3), (3, 6), (6, 8))):
                eng = (nc.sync, nc.gpsimd, nc.scalar)[(c + j) % 3]
                eng.dma_start(out=xc[c][:, lo:hi, :],
                              in_=xT_d.ap()[:, c, lo:hi, :])
            if c == 0:
                # wv after the first x column block (needed by vchunk(0) only)
                nc.scalar.dma_start(out=wv, in_=w_d.ap()[:, 0])
        nc.sync.dma_start(out=wk[:, :, 128:512], in_=w_d.ap()[:, 1, :, 128:512])
        nc.gpsimd.dma_start(out=wq[:, :, 128:512], in_=w_d.ap()[:, 2, :, 128:512])
        nc.scalar.dma_start(out=wo_sb, in_=wo_d.ap())

        nc.vector.memset(vt[:, :, :, DH], 1.0)

        # ---- HAM warm-up: garbage MMs bridge the short pre-data window; the
        # first projections then keep the PE busy and the clock-gate warm.
        ps_w0 = utilp.tile([128, 512], F32, tag="u", name="ps_w0")
        ps_w1 = utilp.tile([128, 512], F32, tag="u", name="ps_w1")
        for i in range(28):
            wb = 64 * (i % 2)
            nc.tensor.matmul(ps_w0 if wb == 0 else ps_w1,
                             kTs[0][wb:wb + 64, 0:128],
                             kTs[0][wb:wb + 64, 0:512],
                             start=True, stop=True, tile_position=(wb, 0))

        # ---- projection emitters (PE work chunks; all write via util pool) ----
        def kq_half(w, dst, s, c, half, cell):
            """4 accumulation MMs; both halves share one PSUM tile via `cell`;
            half 1 finishes the group + copies out."""
            if half == 0:
                cell["ps"] = utilp.tile([128, 512], F32, tag="u",
                                        name=f"kq{s}_{c}_{w is wq}")
            ps = cell["ps"]
            for kt in range(4 * half, 4 * half + 4):
                nc.tensor.matmul(ps, w[:, kt, 128 * s:128 * (s + 1)],
                                 xc[c][:, kt, :],
                                 start=(kt == 0), stop=(kt == KT - 1))
            if half == 1:
                nc.vector.tensor_copy(out=dst[s][:, 512 * c:512 * (c + 1)], in_=ps)

        def kq_chunk(w, dst, s, c):
            cell = {}
            kq_half(w, dst, s, c, 0, cell)
            kq_half(w, dst, s, c, 1, cell)

        def vchunk(t):
            """v projection for token tile t, all 8 heads (512-wide moving)."""
            ps = utilp.tile([128, 512], F32, tag="u", name=f"vps{t}")
            for kt in range(KT):
                nc.tensor.matmul(
                    ps, xc[t // 4][:, kt, 128 * (t % 4):128 * (t % 4 + 1)],
                    wv[:, kt, :],
                    start=(kt == 0), stop=(kt == KT - 1))
            nc.vector.tensor_copy(
                out=vt[:, t, :, 0:DH],
                in_=ps.rearrange("p (h d) -> p h d", h=HPC))

        def oproj_half(ns, c):
            """output projection for token rows 128*ns.., output cols 512*c.."""
            po = utilp.tile([128, 512], F32, tag="u", name=f"po{ns}_{c}")
            for hp in range(NPAIR):
                nc.tensor.matmul(
                    po, aoTs[hp][:, 128 * ns:128 * (ns + 1)],
                    wo_sb[:, hp, 512 * c:512 * (c + 1)],
                    start=(hp == 0), stop=(hp == NPAIR - 1))
            st = stp.tile([128, 512], BF16, tag="st", name=f"st{ns}_{c}")
            nc.vector.tensor_copy(out=st, in_=po)
            nc.sync.dma_start(
                out=out_d.ap()[128 * ns:128 * (ns + 1), 512 * c:512 * (c + 1)],
                in_=st)

        # ---- filler queue: generator items emitting ~460ns sub-chunks ----
        # Items are driven strictly head-first (one active generator at a
        # time), so a multi-sub item's open PSUM accumulation group is never
        # interleaved with another item's util-pool allocation. The queue is
        # pushed in non-decreasing deadline order; oproj items (deadline 98)
        # are appended at the end as their inputs complete.
        fillers = []   # entries: [subcost, deadline, generator]
        reserve = []   # oproj generators held back for the tail bridge
        pending = []   # oproj items awaiting their release block

        def kq_gen(w, dst, s, c):
            ps = utilp.tile([128, 512], F32, tag="u", name=f"kq{s}_{c}_{w is wq}")
            for kt in range(KT):
                nc.tensor.matmul(ps, w[:, kt, 128 * s:128 * (s + 1)],
                                 xc[c][:, kt, :],
                                 start=(kt == 0), stop=(kt == KT - 1))
                if kt % 2 == 1:
                    if kt == KT - 1:
                        nc.vector.tensor_copy(
                            out=dst[s][:, 512 * c:512 * (c + 1)], in_=ps)
                    yield

        def oproj_gen(ns, c, tail=False):
            po = utilp.tile([128, 512], F32, tag="u", name=f"po{ns}_{c}")
            for hp in range(NPAIR):
                nc.tensor.matmul(
                    po, aoTs[hp][:, 128 * ns:128 * (ns + 1)],
                    wo_sb[:, hp, 512 * c:512 * (c + 1)],
                    start=(hp == 0), stop=(hp == NPAIR - 1))
                if hp == 1:
                    yield
            # tail stores ride the (then idle) scalar queue + a deep staging
            # pool so they never contend with the final normalize DMA chains
            pool = stpt if tail else stp
            st = pool.tile([128, 512], BF16, tag="st", name=f"st{ns}_{c}")
            nc.vector.tensor_copy(out=st, in_=po)
            eng = nc.scalar if tail else nc.sync
            eng.dma_start(
                out=out_d.ap()[128 * ns:128 * (ns + 1), 512 * c:512 * (c + 1)],
                in_=st)
            yield

        def push_kq(w, dst, s, c, deadline):
            fillers.append([460, deadline, kq_gen(w, dst, s, c)])

        def emit_one_sub():
            """advance the head generator by one sub-chunk; returns its cost"""
            while fillers:
                item = fillers[0]
                try:
                    next(item[2])
                    return item[0]
                except StopIteration:
                    fillers.pop(0)
            return None

        # block order: s0 pass (blocks 0-3), s1 pass (4-7), then s2/s3
        # interleaved by qb (8-15) so oproj(qb) unblocks early.
        # first-block index per (s, qb):
        def blk_of(s, qb):
            if s < 2:
                return 4 * s + qb
            return 8 + 2 * qb + (s - 2)

        # q chunks c=1..3 for s=0 are needed by blocks 1,2,3
        for c in range(1, NQB):
            push_kq(wq, qTs, 0, c, blk_of(0, c))
        for s in range(1, NPAIR):
            for c in range(NQB):
                push_kq(wk, kTs, s, c, blk_of(s, 0))
            push_kq(wq, qTs, s, 0, blk_of(s, 0))
            for c in range(1, NQB):
                push_kq(wq, qTs, s, c, blk_of(s, c))
        # oproj items are appended dynamically once block (3, qb) completes.

        budget = [0.0]
        STEP_COST = 740.0
        ACT_STEP = 1147.0

        def force_drain(blk_idx):
            # emit everything this block (or earlier) depends on; the queue is
            # deadline-sorted so head-first driving is sufficient
            while fillers and fillers[0][1] <= blk_idx:
                item = fillers[0]
                try:
                    next(item[2])
                    budget[0] -= item[0]
                except StopIteration:
                    fillers.pop(0)

        def pop_budget():
            while fillers and budget[0] >= fillers[0][0]:
                cost = emit_one_sub()
                if cost is None:
                    break
                budget[0] -= cost

        # ---- attention block ----
        def block(s, qb, blk_idx, jit_v=False):
            # fillers this block depends on (k/q chunks) MUST be emitted
            # before the block's first score matmul, or the PE FIFO deadlocks
            budget[0] = max(budget[0], -4000.0)
            force_drain(blk_idx)
            h0, h1 = 2 * s, 2 * s + 1
            av0 = avp.tile([DH + 1, QB], F32, tag="av", name=f"av0_{s}_{qb}")
            av1 = avp.tile([DH + 1, QB], F32, tag="av", name=f"av1_{s}_{qb}")
            pend = []

            def emit_av(t, at):
                nc.tensor.matmul(av0, vt[:, t, h0, :], at[:, 0:QB],
                                 start=(t == 0), stop=(t == NT - 1))
                nc.tensor.matmul(av1, vt[:, t, h1, :], at[:, QB:2 * QB],
                                 start=(t == 0), stop=(t == NT - 1))

            for t in range(NT):
                sc = scp.tile([128, 2 * QB], F32, tag="sc",
                              name=f"sc{s}_{qb}_{t}")
                # both heads' scores back-to-back -> disjoint row groups run
                # concurrently on the PE
                nc.tensor.matmul(sc[:, 0:QB],
                                 kTs[s][0:64, 128 * t:128 * (t + 1)],
                                 qTs[s][0:64, QB * qb:QB * (qb + 1)],
                                 start=True, stop=True, tile_position=(0, 0))
                nc.tensor.matmul(sc[:, QB:2 * QB],
                                 kTs[s][64:128, 128 * t:128 * (t + 1)],
                                 qTs[s][64:128, QB * qb:QB * (qb + 1)],
                                 start=True, stop=True, tile_position=(64, 0))
                at = atp.tile([128, 2 * QB], BF16, tag="at",
                              name=f"at{s}_{qb}_{t}")
                nc.scalar.activation(out=at, in_=sc,
                                     func=mybir.ActivationFunctionType.Exp,
                                     scale=SCALE)
                # fillers slot in here: the PE would otherwise idle waiting
                # for exp(t-2) to finish before the lag-2 AV can consume it
                if not jit_v:
                    budget[0] += ACT_STEP - STEP_COST
                    pop_budget()
                # AV lags two steps: its `at` input was finished by the exp
                # one full step ago, so the PE never waits on the ACT engine
                if len(pend) == 2:
                    emit_av(*pend.pop(0))
                pend.append((t, at))
                if jit_v:
                    # v projection for tile t lands well before its AV
                    vchunk(t)
                    if t % 4 == 3 and t < 12:
                        # next k column block, ahead of its score deadline
                        kq_chunk(wk, kTs, 0, t // 4 + 1)
            while pend:
                emit_av(*pend.pop(0))
            for p in (1, 0):
                normalize(s, qb, p, (av0, av1)[p])

        def normalize(s, qb, p, av):
            sfx = f"{s}_{qb}_{p}"
            # p1 chain rides gpsimd, p0 rides sync: the two per-block chains
            # (3 serial DMA hops each) run in parallel instead of queueing
            dma = nc.gpsimd.dma_start if p == 1 else nc.sync.dma_start
            avu = avup.tile([DH + 1, QB], F32, tag="avu", name=f"avu{sfx}")
            nc.vector.tensor_copy(out=avu, in_=av)
            # spread the denominator over 64 partitions so the reciprocal runs
            # 64-wide instead of 8 cycles/elem on a single partition
            dsp = rcp.tile([DH, QB // DH], F32, tag="dsp", name=f"dsp{sfx}")
            dma(out=dsp, in_=avu[DH:DH + 1, :])
            rc = rcp.tile([DH, QB // DH], F32, tag="rc", name=f"rc{sfx}")
            nc.vector.reciprocal(out=rc, in_=dsp)
            dr = drbp.tile([QB], F32, tag="dr", name=f"dr{sfx}")
            dr_sq = bass.AP(tensor=dr.tensor, offset=dr.offset,
                            ap=[[QB // DH, DH], [1, QB // DH]])
            dma(out=dr_sq, in_=rc)
            dr_bc = bass.AP(tensor=dr.tensor, offset=dr.offset,
                            ap=[[0, DH]] + [list(dd) for dd in dr.ap])
            bcs = bcsp.tile([DH, QB], F32, tag="bcs", name=f"bcs{sfx}")
            dma(out=bcs, in_=dr_bc)
            if p == 0:
                with nc.allow_low_precision(reason="attn out in bf16"):
                    nc.vector.tensor_mul(
                        out=aoTs[s][0:DH, QB * qb:QB * (qb + 1)],
                        in0=avu[0:DH, :], in1=bcs)
            else:
                od = oddp.tile([DH, QB], BF16, tag="od", name=f"od{sfx}")
                with nc.allow_low_precision(reason="attn out in bf16"):
                    nc.vector.tensor_mul(out=od, in0=avu[0:DH, :], in1=bcs)
                nc.gpsimd.dma_start(
                    out=aoTs[s][DH:128, QB * qb:QB * (qb + 1)], in_=od)

        # ---- pre-phase: first k/q chunks for block (0,0) ----
        kq_chunk(wk, kTs, 0, 0)
        kq_chunk(wq, qTs, 0, 0)

        # ---- main loop: s0 pass, s1 pass, then s2/s3 interleaved by qb ----
        order = ([(0, qb) for qb in range(NQB)] +
                 [(1, qb) for qb in range(NQB)] +
                 [(s, qb) for qb in range(NQB) for s in (2, 3)])
        for blk, (s, qb) in enumerate(order):
            while pending and pending[0][0] <= blk - 2:
                src_blk, cost, gen = pending.pop(0)
                fillers.append([cost, 98, gen])
            block(s, qb, blk, jit_v=(blk == 0))
            if s == NPAIR - 1 and qb < NQB - 1:
                # aoT rows for this qb now complete -> queue output proj.
                # qb0/qb1 go to `pending` (released one full block later so
                # their first matmul never waits on this block's od-DMA and
                # stalls the PE FIFO); all of qb2 is reserved as tail bridge
                # work to keep the PE busy+warm across the final normalize.
                for ns in range(4 * qb, 4 * qb + 4):
                    for c in range(2):
                        if qb == NQB - 2:
                            reserve.append(oproj_gen(ns, c, tail=True))
                        else:
                            pending.append([blk, 640, oproj_gen(ns, c)])

        # ---- tail: drain leftover fillers, then the last qb's output
        # projection with its hp=0..2 accumulation prefix emitted DURING the
        # final normalize (only the hp=3 matmul + cast + store depend on it).
        # The prefix MMs double as HAM warm-guards across the normalize wait.
        for src_blk, cost, gen in pending:
            fillers.append([cost, 98, gen])
        pending.clear()
        while emit_one_sub() is not None:
            pass
        for gen in reserve:
            for _ in gen:
                pass
        # short garbage burst: keeps the PE busy (and the clock-gate warm)
        # through the remainder of the final normalize's DMA chain
        for i in range(18):
            wb = 64 * (i % 2)
            ps_wg = utilp.tile([128, 512], F32, tag="u", name=f"twg{i}")
            nc.tensor.matmul(ps_wg, kTs[0][wb:wb + 64, 0:128],
                             kTs[0][wb:wb + 64, 0:512],
                             start=True, stop=True, tile_position=(wb, 0))
        tail_halves = [(ns, c) for ns in range(4 * (NQB - 1), 4 * NQB)
                       for c in range(2)]
        slots = []
        for i in range(2):
            sct = scp.tile([128, 2 * QB], F32, tag="sc", name=f"tpo{i}")
            slots += [sct[:, 0:512], sct[:, 512:1024]]
        for i in range(2):
            slots.append(utilp.tile([128, 512], F32, tag="u", name=f"tpo_u{i}"))
        for i in range(2):
            slots.append(avp.tile([128, 512], F32, tag="av", name=f"tpo_a{i}"))
        for idx, (ns, c) in enumerate(tail_halves):
            for hp in range(NPAIR - 1):
                nc.tensor.matmul(
                    slots[idx], aoTs[hp][:, 128 * ns:128 * (ns + 1)],
                    wo_sb[:, hp, 512 * c:512 * (c + 1)],
                    start=(hp == 0), stop=False)
        for idx, (ns, c) in enumerate(tail_halves):
            nc.tensor.matmul(
                slots[idx], aoTs[NPAIR - 1][:, 128 * ns:128 * (ns + 1)],
                wo_sb[:, NPAIR - 1, 512 * c:512 * (c + 1)],
                start=False, stop=True)
            st = stpt.tile([128, 512], BF16, tag="st", name=f"tst{ns}_{c}")
            nc.vector.tensor_copy(out=st, in_=slots[idx])
            eng = (nc.scalar, nc.sync)[idx % 2]
            eng.dma_start(
                out=out_d.ap()[128 * ns:128 * (ns + 1), 512 * c:512 * (c + 1)],
                in_=st)


def get_program():
    if "nc" not in _CACHE:
        _CACHE["nc"] = _build_program()
    return _CACHE["nc"]


def make_in_maps(x, w_qkv, w_out, b_out):
    bf = ml_dtypes.bfloat16
    x = np.asarray(x, np.float32)
    w_qkv = np.asarray(w_qkv, np.float32)
    w_out = np.asarray(w_out, np.float32)
    b_out = np.asarray(b_out, np.float32)

    in_maps = []
    for core in range(NCORES):
        b, hh = core // 2, core % 2
        # xT in [128, NQB, KT, 512] layout: [p, c, t, e] = x[b].T[128t+p, 512c+e]
        # (column blocks contiguous so each input DMA is a dense 3KB+/partition
        # transfer instead of strided 1KB rows)
        xT = np.ascontiguousarray(x[b].T).astype(bf)                 # [DIM, N]
        xT_pt = np.ascontiguousarray(
            xT.reshape(KT, 128, NQB, 512).transpose(1, 2, 0, 3))
        # w slices for this head-half, groups ordered [v, k, q]
        wq = w_qkv[:, 512 * hh:512 * (hh + 1)]
        wk = w_qkv[:, DIM + 512 * hh:DIM + 512 * (hh + 1)]
        wv = w_qkv[:, 2 * DIM + 512 * hh:2 * DIM + 512 * (hh + 1)]
        wcat = np.stack([wv, wk, wq], axis=0).astype(bf)             # [3, DIM, 512]
        w_pt = np.ascontiguousarray(
            wcat.reshape(3, KT, 128, 512).transpose(2, 0, 1, 3))    # [p, g, t, e]
        # w_out rows for this half -> [p, hp, d]
        wo = w_out[512 * hh:512 * (hh + 1), :].astype(bf)            # [512, DIM]
        wo_pt = np.ascontiguousarray(wo.reshape(NPAIR, 128, DIM).transpose(1, 0, 2))
        in_maps.append({
            "xT": xT_pt,
            "w_qkv": w_pt,
            "w_out": wo_pt,
        })
    return in_maps


def kernel(x, w_qkv, w_out, b_out):
    nc = get_program()
    in_maps = make_in_maps(x, w_qkv, w_out, b_out)
    res = bass_utils.run_bass_kernel_spmd(nc, in_maps, core_ids=list(range(NCORES)))
    out = np.empty((B, N, DIM), np.float32)
    bias = np.asarray(b_out, np.float32)
    for b in range(B):
        out[b] = np.asarray(res.results[2 * b]["out"], np.float32)
        out[b] += np.asarray(res.results[2 * b + 1]["out"], np.float32)
        out[b] += bias
    return out
